# revision 11
# baseline (speedup 1.0000x reference)
"""MoE top-2 routing kernel for 8 Trainium2 NeuronCores.

Problem: x[2,4096,1024] tokens, 8 experts W[8,1024,1024]+b[8,1024],
top-2 expert indices + gate weights per token.
out[t] = sum_k gate[t,k] * (x[t] @ W[idx[t,k]] + b[idx[t,k]])

Strategy (data-parallel dispatch):
- Flatten tokens to [8192, 1024]; core c owns tokens [c*1024,(c+1)*1024).
- Host computes routing from the (input) indices: per expert, the list of
  (local token, gate), same-expert duplicates merged (gates summed), each
  expert segment padded to a multiple of 128 rows (pad = token 0, gate 0).
  Per-expert tile capacities are maxed across cores so all 8 cores run one
  SPMD program.
- On-chip per expert: dma_gather(transpose=True) pulls the routed token rows
  of x (fp16) from DRAM directly into the transposed [128d x ntok] layout the
  PE needs; 128-token tiles are matmul'd against W_e (fp16, f32 PSUM
  accumulation over 8 K-chunks) with the bias added via a ones-row matmul;
  DVE scales rows by the gate; dma_scatter_add accumulates rows into the
  zero-initialized f32 output.
- fp16 keeps absmax error ~3e-4 of output scale (vs 2e-3 for bf16) at
  identical PE throughput.
"""

import os
import sys

import numpy as np

for _p in ("/opt/trn_rl_repo", os.path.expanduser("~/.axon_site/_ro/trn_rl_repo")):
    if os.path.isdir(_p) and _p not in sys.path:
        sys.path.insert(0, _p)

B, S, D, E, K = 2, 4096, 1024, 8, 2
N_CORES = 8
TOKENS = B * S
TOK_PER_CORE = TOKENS // N_CORES  # 1024
P = 128
DCHUNKS = D // P  # 8
FH = 512  # psum bank half of D
NH = D // FH  # 2


def _build_routing(top_k_indices, expert_weights):
    """Per-core, per-expert token/gate lists (deduped) + shared tile caps."""
    idx = np.asarray(top_k_indices).reshape(-1, K)
    gw = np.asarray(expert_weights).reshape(-1, K).astype(np.float32)
    per_core = []
    for c in range(N_CORES):
        lo = c * TOK_PER_CORE
        lists = [[] for _ in range(E)]
        for t in range(TOK_PER_CORE):
            e0 = int(idx[lo + t, 0])
            e1 = int(idx[lo + t, 1])
            g0 = float(gw[lo + t, 0])
            g1 = float(gw[lo + t, 1])
            if e0 == e1:
                lists[e0].append((t, g0 + g1))
            else:
                lists[e0].append((t, g0))
                lists[e1].append((t, g1))
        per_core.append(lists)
    caps = [
        max(1, max((len(per_core[c][e]) + P - 1) // P for c in range(N_CORES)))
        for e in range(E)
    ]
    return per_core, caps


def _wrap_idxs(idx_disp):
    # dma_gather/scatter idx layout: logical i lives at [i % 16, i // 16],
    # replicated into all eight 16-partition groups (each GpSimd Q7 core
    # reads the group at [16*cpu_id, 16*cpu_id + 16)).
    tot = idx_disp.size
    return np.tile(idx_disp.reshape(tot // 16, 16).T, (P // 16, 1))


def _build_dispatch(lists, caps):
    """Dispatch-order gather/scatter index (int16) and gate (f32) arrays."""
    gidx_disp = []  # gather: dummy rows read x row 0 (real data, gate 0)
    sidx_disp = []  # scatter: dummy rows add their (zero) output to a trash row
    gate_disp = []
    for e in range(E):
        n = caps[e] * P
        ent = lists[e]
        assert len(ent) <= n
        pad = n - len(ent)
        gidx_disp += [t for t, _ in ent] + [0] * pad
        sidx_disp += [t for t, _ in ent] + [TOK_PER_CORE] * pad
        gate_disp += [g for _, g in ent] + [0.0] * pad
    gidx_disp = np.asarray(gidx_disp, np.int16)
    sidx_disp = np.asarray(sidx_disp, np.int16)
    gate_disp = np.asarray(gate_disp, np.float32)
    tot = gate_disp.size
    # gate layout: dispatch row i at [i % 128, i // 128]
    gates_sb = np.ascontiguousarray(gate_disp.reshape(tot // P, P).T)
    return _wrap_idxs(gidx_disp), _wrap_idxs(sidx_disp), gates_sb


def _build_program(caps):
    import concourse.bass as bass  # noqa: F401
    import concourse.tile as tile
    from concourse import bacc, mybir

    fp16 = mybir.dt.float16
    f32 = mybir.dt.float32
    i16 = mybir.dt.int16

    tot = sum(caps) * P
    nc = bacc.Bacc("TRN2", target_bir_lowering=False, debug=False)

    x_d = nc.dram_tensor("x", [TOK_PER_CORE, D], fp16, kind="ExternalInput").ap()
    w_d = nc.dram_tensor("w", [E * P, DCHUNKS * D], fp16, kind="ExternalInput").ap()
    b_d = nc.dram_tensor("b", [1, E * D], fp16, kind="ExternalInput").ap()
    idx_d = nc.dram_tensor("idxs", [P, tot // 16], i16, kind="ExternalInput").ap()
    sidx_d = nc.dram_tensor("sidxs", [P, tot // 16], i16, kind="ExternalInput").ap()
    gate_d = nc.dram_tensor("gates", [P, tot // P], f32, kind="ExternalInput").ap()
    # +8 trash rows: dummy (padding) scatter rows accumulate their exact-zero
    # payload there instead of racing real rows.
    out_d = nc.dram_tensor("out", [TOK_PER_CORE + 8, D], f32, kind="ExternalOutput").ap()

    with tile.TileContext(nc) as tc:
        with (
            tc.tile_pool(name="const", bufs=1) as cpool,
            tc.tile_pool(name="wpool", bufs=2) as wpool,
            tc.tile_pool(name="xgpool", bufs=2) as xgpool,
            tc.tile_pool(name="ypool", bufs=2) as ypool,
            tc.tile_pool(name="pspool", bufs=2, space="PSUM") as pspool,
        ):
            ones_sb = cpool.tile([1, P], fp16)
            nc.vector.memset(ones_sb[:], 1.0)
            b_sb = cpool.tile([1, E * D], fp16)
            nc.sync.dma_start(b_sb[:], b_d[:])
            idx_sb = cpool.tile([P, tot // 16], i16)
            nc.sync.dma_start(idx_sb[:], idx_d[:])
            sidx_sb = cpool.tile([P, tot // 16], i16)
            nc.sync.dma_start(sidx_sb[:], sidx_d[:])
            gate_sb = cpool.tile([P, tot // P], f32)
            nc.sync.dma_start(gate_sb[:], gate_d[:])

            # Zero-init the output (scatter_add accumulates into it).
            zero_sb = cpool.tile([P, D], f32)
            nc.vector.memset(zero_sb[:], 0.0)
            for r in range(TOK_PER_CORE // P):
                nc.sync.dma_start(out_d[r * P : (r + 1) * P, :], zero_sb[:])

            tile_off = 0
            for e in range(E):
                ce = caps[e]
                n_e = ce * P
                w_sb = wpool.tile([P, DCHUNKS, D], fp16, tag="w", name="w_sb")
                nc.sync.dma_start(
                    w_sb[:], w_d[e * P : (e + 1) * P, :].rearrange("p (c d) -> p c d", c=DCHUNKS)
                )
                xg = xgpool.tile([P, DCHUNKS, n_e], fp16, tag="xg", name="xg")
                col0 = tile_off * (P // 16)
                nc.gpsimd.dma_gather(
                    xg[:],
                    x_d[:],
                    idx_sb[:, col0 : col0 + n_e // 16],
                    n_e,
                    n_e,
                    D,
                    transpose=True,
                )
                y_sb = ypool.tile([P, ce, D], f32, tag="y", name="y_sb")
                for t in range(ce):
                    ps = pspool.tile([P, D], f32, tag="ps", name="ps")
                    for c in range(DCHUNKS + 1):
                        for h in range(NH):
                            if c < DCHUNKS:
                                lhsT = xg[:, c, t * P : (t + 1) * P]
                                rhs = w_sb[:, c, h * FH : (h + 1) * FH]
                            else:
                                lhsT = ones_sb[0:1, :]
                                rhs = b_sb[0:1, e * D + h * FH : e * D + (h + 1) * FH]
                            nc.tensor.matmul(
                                ps[:, h * FH : (h + 1) * FH],
                                lhsT,
                                rhs,
                                start=(c == 0),
                                stop=(c == DCHUNKS),
                            )
                    gt = tile_off + t
                    nc.vector.tensor_scalar_mul(
                        y_sb[:, t, :], ps[:, :], gate_sb[:, gt : gt + 1]
                    )
                nc.gpsimd.dma_scatter_add(
                    out_d[:],
                    y_sb[:],
                    sidx_sb[:, col0 : col0 + n_e // 16],
                    n_e,
                    n_e,
                    D,
                )
                tile_off += ce
    nc.compile()
    return nc


def _prep_inputs(x, expert_weights, top_k_indices, W, b):
    """Host-side sharding: per-core input maps + caps."""
    per_core, caps = _build_routing(top_k_indices, expert_weights)
    x_flat = np.asarray(x, np.float32).reshape(TOKENS, D)
    w_hw = np.ascontiguousarray(
        np.asarray(W, np.float32)
        .reshape(E, DCHUNKS, P, D)
        .transpose(0, 2, 1, 3)
        .astype(np.float16)
        .reshape(E * P, DCHUNKS * D)
    )
    b_hw = np.ascontiguousarray(np.asarray(b, np.float32).astype(np.float16).reshape(1, E * D))
    in_maps = []
    for c in range(N_CORES):
        idxs_sb, sidxs_sb, gates_sb = _build_dispatch(per_core[c], caps)
        xc = np.ascontiguousarray(
            x_flat[c * TOK_PER_CORE : (c + 1) * TOK_PER_CORE].astype(np.float16)
        )
        in_maps.append(
            {
                "x": xc,
                "w": w_hw,
                "b": b_hw,
                "idxs": idxs_sb,
                "sidxs": sidxs_sb,
                "gates": gates_sb,
            }
        )
    return in_maps, caps


def kernel(x, expert_weights, top_k_indices, W, b):
    from concourse.bass_utils import run_bass_kernel_spmd

    in_maps, caps = _prep_inputs(x, expert_weights, top_k_indices, W, b)
    nc = _build_program(caps)
    res = run_bass_kernel_spmd(
        nc,
        in_maps,
        core_ids=list(range(N_CORES)),
        trace=bool(int(os.environ.get("KERNEL_TRACE", "0"))),
    )
    out = np.concatenate([r["out"][:TOK_PER_CORE] for r in res.results], axis=0)
    if bool(int(os.environ.get("KERNEL_TRACE", "0"))):
        kernel.last_results = res
    return np.ascontiguousarray(out.reshape(B, S, D).astype(np.float32))


# revision 14
# speedup vs baseline: 1.0356x; 1.0356x over previous
"""MoE top-2 routing kernel for 8 Trainium2 NeuronCores.

Problem: x[2,4096,1024] tokens, 8 experts W[8,1024,1024]+b[8,1024],
top-2 expert indices + gate weights per token.
out[t] = sum_k gate[t,k] * (x[t] @ W[idx[t,k]] + b[idx[t,k]])

Strategy (data-parallel dispatch):
- Flatten tokens to [8192, 1024]; core c owns tokens [c*1024,(c+1)*1024).
- Host computes routing from the (input) indices: per expert, the list of
  (local token, gate), same-expert duplicates merged (gates summed), each
  expert segment padded to a multiple of 128 rows (pad = token 0, gate 0).
  Per-expert tile capacities are maxed across cores so all 8 cores run one
  SPMD program.
- On-chip per expert: dma_gather(transpose=True) pulls the routed token rows
  of x (fp16) from DRAM directly into the transposed [128d x ntok] layout the
  PE needs; 128-token tiles are matmul'd against W_e (fp16, f32 PSUM
  accumulation over 8 K-chunks) with the bias added via a ones-row matmul;
  DVE scales rows by the gate; dma_scatter_add accumulates rows into the
  zero-initialized f32 output.
- fp16 keeps absmax error ~3e-4 of output scale (vs 2e-3 for bf16) at
  identical PE throughput.
"""

import os
import sys

import numpy as np

for _p in ("/opt/trn_rl_repo", os.path.expanduser("~/.axon_site/_ro/trn_rl_repo")):
    if os.path.isdir(_p) and _p not in sys.path:
        sys.path.insert(0, _p)

B, S, D, E, K = 2, 4096, 1024, 8, 2
N_CORES = 8
TOKENS = B * S
TOK_PER_CORE = TOKENS // N_CORES  # 1024
P = 128
DCHUNKS = D // P  # 8
FH = 512  # psum bank half of D
NH = D // FH  # 2


def _build_routing(top_k_indices, expert_weights):
    """Per-core, per-expert token/gate lists (deduped) + shared tile caps."""
    idx = np.asarray(top_k_indices).reshape(-1, K)
    gw = np.asarray(expert_weights).reshape(-1, K).astype(np.float32)
    per_core = []
    for c in range(N_CORES):
        lo = c * TOK_PER_CORE
        lists = [[] for _ in range(E)]
        for t in range(TOK_PER_CORE):
            e0 = int(idx[lo + t, 0])
            e1 = int(idx[lo + t, 1])
            g0 = float(gw[lo + t, 0])
            g1 = float(gw[lo + t, 1])
            if e0 == e1:
                lists[e0].append((t, g0 + g1))
            else:
                lists[e0].append((t, g0))
                lists[e1].append((t, g1))
        per_core.append(lists)
    caps = [
        max(1, max((len(per_core[c][e]) + P - 1) // P for c in range(N_CORES)))
        for e in range(E)
    ]
    return per_core, caps


def _wrap_idxs(idx_disp):
    # dma_gather/scatter idx layout: logical i lives at [i % 16, i // 16],
    # replicated into all eight 16-partition groups (each GpSimd Q7 core
    # reads the group at [16*cpu_id, 16*cpu_id + 16)).
    tot = idx_disp.size
    return np.tile(idx_disp.reshape(tot // 16, 16).T, (P // 16, 1))


def _build_dispatch(lists, caps):
    """Dispatch-order gather/scatter index (int16) and gate (f32) arrays."""
    gidx_disp = []  # gather: dummy rows read x row 0 (real data, gate 0)
    sidx_disp = []  # scatter: dummy rows add their (zero) output to a trash row
    gate_disp = []
    for e in range(E):
        n = caps[e] * P
        ent = lists[e]
        assert len(ent) <= n
        pad = n - len(ent)
        gidx_disp += [t for t, _ in ent] + [0] * pad
        sidx_disp += [t for t, _ in ent] + [TOK_PER_CORE] * pad
        gate_disp += [g for _, g in ent] + [0.0] * pad
    gidx_disp = np.asarray(gidx_disp, np.int16)
    sidx_disp = np.asarray(sidx_disp, np.int16)
    gate_disp = np.asarray(gate_disp, np.float32)
    tot = gate_disp.size
    # gate layout: dispatch row i at [i % 128, i // 128]
    gates_sb = np.ascontiguousarray(gate_disp.reshape(tot // P, P).T)
    return _wrap_idxs(gidx_disp), _wrap_idxs(sidx_disp), gates_sb


def _build_program(caps):
    import concourse.bass as bass  # noqa: F401
    import concourse.tile as tile
    from concourse import bacc, mybir

    fp16 = mybir.dt.float16
    f32 = mybir.dt.float32
    i16 = mybir.dt.int16

    tot = sum(caps) * P
    nc = bacc.Bacc("TRN2", target_bir_lowering=False, debug=False)

    x_d = nc.dram_tensor("x", [TOK_PER_CORE, D], fp16, kind="ExternalInput").ap()
    w_d = nc.dram_tensor("w", [E * P, DCHUNKS * D], fp16, kind="ExternalInput").ap()
    b_d = nc.dram_tensor("b", [1, E * D], fp16, kind="ExternalInput").ap()
    idx_d = nc.dram_tensor("idxs", [P, tot // 16], i16, kind="ExternalInput").ap()
    sidx_d = nc.dram_tensor("sidxs", [P, tot // 16], i16, kind="ExternalInput").ap()
    gate_d = nc.dram_tensor("gates", [P, tot // P], f32, kind="ExternalInput").ap()
    # +8 trash rows: dummy (padding) scatter rows accumulate their exact-zero
    # payload there instead of racing real rows.
    out_d = nc.dram_tensor("out", [TOK_PER_CORE + 8, D], f32, kind="ExternalOutput").ap()

    # Process largest experts first so the kernel tail (last expert's
    # epilogue + scatter) is as small as possible.
    order = sorted(range(E), key=lambda e: -caps[e])
    off = [sum(caps[:e]) for e in range(E)]  # tile offset of expert e in dispatch

    with tile.TileContext(nc) as tc:
        with (
            tc.tile_pool(name="const", bufs=1) as cpool,
            tc.tile_pool(name="wpool", bufs=3) as wpool,
            tc.tile_pool(name="xgpool", bufs=3) as xgpool,
            tc.tile_pool(name="ypool", bufs=2) as ypool,
            tc.tile_pool(name="pspool", bufs=2, space="PSUM") as pspool,
        ):
            # Tiny metadata loads first (the first gather needs idx_sb).
            idx_sb = cpool.tile([P, tot // 16], i16)
            nc.sync.dma_start(idx_sb[:], idx_d[:])
            sidx_sb = cpool.tile([P, tot // 16], i16)
            nc.scalar.dma_start(sidx_sb[:], sidx_d[:])
            gate_sb = cpool.tile([P, tot // P], f32)
            nc.scalar.dma_start(gate_sb[:], gate_d[:])
            b_sb = cpool.tile([1, E * D], fp16)
            nc.scalar.dma_start(b_sb[:], b_d[:])
            ones_sb = cpool.tile([1, P], fp16)
            nc.vector.memset(ones_sb[:], 1.0)

            # Software-pipelined prefetch of W (HWDGE/sync ring) and the
            # x row-gather (SWDGE queue 0).
            pref = {}

            def prefetch(k):
                if k >= E:
                    return
                e = order[k]
                n_e = caps[e] * P
                col0 = off[e] * (P // 16)
                w_sb = wpool.tile([P, DCHUNKS, D], fp16, tag="w", name="w_sb")
                nc.sync.dma_start(
                    w_sb[:],
                    w_d[e * P : (e + 1) * P, :].rearrange(
                        "p (c d) -> p c d", c=DCHUNKS
                    ),
                )
                xg = xgpool.tile([P, DCHUNKS, n_e], fp16, tag="xg", name="xg")
                nc.gpsimd.dma_gather(
                    xg[:],
                    x_d[:],
                    idx_sb[:, col0 : col0 + n_e // 16],
                    n_e,
                    n_e,
                    D,
                    transpose=True,
                )
                pref[k] = (w_sb, xg)

            prefetch(0)
            prefetch(1)

            # Zero-init the output (scatter_add accumulates into it) on the
            # scalar HWDGE ring, overlapping the first expert's compute.
            zero_sb = cpool.tile([P, D], f32)
            nc.vector.memset(zero_sb[:], 0.0)
            for r in range(TOK_PER_CORE // P):
                nc.scalar.dma_start(out_d[r * P : (r + 1) * P, :], zero_sb[:])

            for k in range(E):
                e = order[k]
                ce = caps[e]
                n_e = ce * P
                col0 = off[e] * (P // 16)
                w_sb, xg = pref.pop(k)
                y_sb = ypool.tile([P, ce, D], f32, tag="y", name="y_sb")
                for t in range(ce):
                    ps = pspool.tile([P, D], f32, tag="ps", name="ps")
                    for c in range(DCHUNKS + 1):
                        for h in range(NH):
                            if c < DCHUNKS:
                                lhsT = xg[:, c, t * P : (t + 1) * P]
                                rhs = w_sb[:, c, h * FH : (h + 1) * FH]
                            else:
                                lhsT = ones_sb[0:1, :]
                                rhs = b_sb[0:1, e * D + h * FH : e * D + (h + 1) * FH]
                            nc.tensor.matmul(
                                ps[:, h * FH : (h + 1) * FH],
                                lhsT,
                                rhs,
                                start=(c == 0),
                                stop=(c == DCHUNKS),
                            )
                    gt = off[e] + t
                    nc.vector.tensor_scalar_mul(
                        y_sb[:, t, :], ps[:, :], gate_sb[:, gt : gt + 1]
                    )
                # Prefetch k+2 BEFORE the scatter so the gather prep isn't
                # stuck behind the scatter's epilogue-wait on the gpsimd
                # queue (and its ring bytes drain ahead of the scatter's).
                prefetch(k + 2)
                nc.gpsimd.dma_scatter_add(
                    out_d[:],
                    y_sb[:],
                    sidx_sb[:, col0 : col0 + n_e // 16],
                    n_e,
                    n_e,
                    D,
                )
    nc.compile()
    return nc


def _prep_inputs(x, expert_weights, top_k_indices, W, b):
    """Host-side sharding: per-core input maps + caps."""
    per_core, caps = _build_routing(top_k_indices, expert_weights)
    x_flat = np.asarray(x, np.float32).reshape(TOKENS, D)
    w_hw = np.ascontiguousarray(
        np.asarray(W, np.float32)
        .reshape(E, DCHUNKS, P, D)
        .transpose(0, 2, 1, 3)
        .astype(np.float16)
        .reshape(E * P, DCHUNKS * D)
    )
    b_hw = np.ascontiguousarray(np.asarray(b, np.float32).astype(np.float16).reshape(1, E * D))
    in_maps = []
    for c in range(N_CORES):
        idxs_sb, sidxs_sb, gates_sb = _build_dispatch(per_core[c], caps)
        xc = np.ascontiguousarray(
            x_flat[c * TOK_PER_CORE : (c + 1) * TOK_PER_CORE].astype(np.float16)
        )
        in_maps.append(
            {
                "x": xc,
                "w": w_hw,
                "b": b_hw,
                "idxs": idxs_sb,
                "sidxs": sidxs_sb,
                "gates": gates_sb,
            }
        )
    return in_maps, caps


def kernel(x, expert_weights, top_k_indices, W, b):
    from concourse.bass_utils import run_bass_kernel_spmd

    in_maps, caps = _prep_inputs(x, expert_weights, top_k_indices, W, b)
    nc = _build_program(caps)
    res = run_bass_kernel_spmd(
        nc,
        in_maps,
        core_ids=list(range(N_CORES)),
        trace=bool(int(os.environ.get("KERNEL_TRACE", "0"))),
    )
    out = np.concatenate([r["out"][:TOK_PER_CORE] for r in res.results], axis=0)
    if bool(int(os.environ.get("KERNEL_TRACE", "0"))):
        kernel.last_results = res
    return np.ascontiguousarray(out.reshape(B, S, D).astype(np.float32))
